# revision 2
# baseline (speedup 1.0000x reference)
"""Trainium2 Bass kernel for nn_PoseODE (RK4(3/8) neural-ODE + regressor).

Structure of the reference:
  - fused = cat(fv, fi) is used ONLY for its shape; the ODE state starts at
    y0 = zeros and f(y) = tanh(y@W1+b1)@W2+b2 never reads the batch data.
    Every batch row therefore carries the identical trajectory -> compute a
    single row on-device and broadcast on the host. This is exact for any
    input values.
  - RK4(3/8) with state y and a := y@W1+b1 carried instead of y:
        s1 = tanh(a)
        u2 = a + (dt/3)(s1@W21 + c)        s2 = tanh(u2)
        u3 = a + dt((s2-s1/3)@W21 + 2c/3)  s3 = tanh(u3)
        u4 = a + dt((s1-s2+s3)@W21 + c)    s4 = tanh(u4)
        v4 = s1+3(s2+s3)+s4;  a' = a + (dt/8)(v4@W21 + 8c);  G += (dt/8) v4
    with W21 = W2@W1, c = b2@W1 (host-precomputed). After S steps:
        yT = G@W2 + (sum dt) b2, then the 768->128->256->128->6 LeakyReLU
    regressor. This halves the sequential matmul count (4/step instead of 8).

  - On-device layout is feature-major: a 768-vector lives in an SBUF tile
    [128 partitions, 6 cols]. Each matvec is 36 fp32 PE matmuls (6 j-tiles x
    6 k-tiles of the stationary 128x128 weight block, moving operand = one
    state column, N=1), accumulated per j-column in a single PSUM bank
    (sequential start/stop groups per column preserve earlier columns' data).
"""

import os
import numpy as np

F = 768
KT = 6  # 768 / 128
S_DEFAULT = 64
N_CORES = 8

_CACHE: dict = {}


def _build(dts: tuple) -> "object":
    import concourse.bacc as bacc
    import concourse.bass as bass
    import concourse.mybir as mybir
    from concourse.tile import TileContext

    f32 = mybir.dt.float32
    Alu = mybir.AluOpType
    Act = mybir.ActivationFunctionType
    S = len(dts)

    nc = bacc.Bacc("TRN2", target_bir_lowering=False, debug=False)

    w21_d = nc.dram_tensor("w21", [F, F], f32, kind="ExternalInput").ap()
    w2_d = nc.dram_tensor("w2b", [F, F], f32, kind="ExternalInput").ap()
    rw1_d = nc.dram_tensor("rw1", [F, 128], f32, kind="ExternalInput").ap()
    rw2_d = nc.dram_tensor("rw2", [128, 256], f32, kind="ExternalInput").ap()
    rw3_d = nc.dram_tensor("rw3", [256, 128], f32, kind="ExternalInput").ap()
    rw4_d = nc.dram_tensor("rw4", [128, 6], f32, kind="ExternalInput").ap()
    sm_d = nc.dram_tensor("smalls", [128, 48], f32, kind="ExternalInput").ap()
    yt_d = nc.dram_tensor("yt", [F], f32, kind="ExternalOutput").ap()
    pose_d = nc.dram_tensor("pose", [6], f32, kind="ExternalOutput").ap()

    with TileContext(nc) as tc:
        with (
            tc.tile_pool(name="w", bufs=1) as wpool,
            tc.tile_pool(name="st", bufs=1) as spool,
            tc.tile_pool(name="ps", bufs=8, space="PSUM") as ppool,
        ):
            # --- constant loads -------------------------------------------------
            w21_s = wpool.tile([128, KT * KT * 128], f32)
            nc.sync.dma_start(
                w21_s[:].rearrange("p (k j m) -> p k j m", k=KT, j=KT),
                w21_d.rearrange("(k p) (j m) -> p k j m", p=128, j=KT),
            )
            sm_s = wpool.tile([128, 48], f32)
            nc.sync.dma_start(sm_s[:], sm_d)
            w2_s = wpool.tile([128, KT * KT * 128], f32)
            nc.sync.dma_start(
                w2_s[:].rearrange("p (k j m) -> p k j m", k=KT, j=KT),
                w2_d.rearrange("(k p) (j m) -> p k j m", p=128, j=KT),
            )
            rw1_s = wpool.tile([128, KT * 128], f32)
            nc.sync.dma_start(
                rw1_s[:].rearrange("p (k m) -> p k m", k=KT),
                rw1_d.rearrange("(k p) m -> p k m", p=128),
            )
            rw2_s = wpool.tile([128, 256], f32)
            nc.sync.dma_start(rw2_s[:], rw2_d)
            rw3_s = wpool.tile([128, 256], f32)
            nc.sync.dma_start(
                rw3_s[:].rearrange("p (k m) -> p k m", k=2),
                rw3_d.rearrange("(k p) m -> p k m", p=128),
            )
            rw4_s = wpool.tile([128, 6], f32)
            nc.sync.dma_start(rw4_s[:], rw4_d)

            CC = sm_s[:, 12:18]
            C23 = sm_s[:, 18:24]
            C8 = sm_s[:, 24:30]
            TB2 = sm_s[:, 30:36]

            # --- persistent state ----------------------------------------------
            a = spool.tile([128, KT], f32)
            s1 = spool.tile([128, KT], f32)
            s2 = spool.tile([128, KT], f32)
            s3 = spool.tile([128, KT], f32)
            s4 = spool.tile([128, KT], f32)
            v2 = spool.tile([128, KT], f32)
            v3 = spool.tile([128, KT], f32)
            v4 = spool.tile([128, KT], f32)
            d12 = spool.tile([128, KT], f32)
            qq = spool.tile([128, KT], f32)
            hh = spool.tile([128, KT], f32)
            G = spool.tile([128, KT], f32)
            ac = spool.tile([128, KT], f32)
            u = spool.tile([128, KT], f32)
            yt_s = spool.tile([128, KT], f32)

            nc.vector.tensor_copy(a[:], sm_s[:, 0:6])
            nc.vector.tensor_copy(s1[:], sm_s[:, 6:12])
            nc.vector.memset(G[:], 0.0)

            def matvec(w_s, v):
                """v[128,6] @ W (36 fp32 matmuls) -> psum tile [128,6]."""
                ps = ppool.tile([128, KT], f32, tag="ps")
                for j in range(KT):
                    for k in range(KT):
                        blk = (k * KT + j) * 128
                        nc.tensor.matmul(
                            ps[:, j : j + 1],
                            w21_s[:, blk : blk + 128] if w_s is None else w_s[:, blk : blk + 128],
                            v[:, k : k + 1],
                            start=(k == 0),
                            stop=(k == KT - 1),
                        )
                return ps

            # --- the 64-step scan ----------------------------------------------
            for t in range(S):
                dt = float(dts[t])
                dt3 = dt / 3.0
                dt8 = dt / 8.0

                # round 1: p1 = s1 @ W21 ; s2 = tanh(dt3*p1 + (a + dt3*c))
                ps = matvec(None, s1)
                nc.vector.scalar_tensor_tensor(ac[:], CC, dt3, a[:], Alu.mult, Alu.add)
                nc.vector.scalar_tensor_tensor(u[:], ps[:], dt3, ac[:], Alu.mult, Alu.add)
                nc.scalar.activation(s2[:], u[:], Act.Tanh)
                nc.vector.scalar_tensor_tensor(
                    v2[:], s1[:], -1.0 / 3.0, s2[:], Alu.mult, Alu.add
                )

                # round 2: p2 = v2 @ W21 ; s3 = tanh(dt*p2 + (a + dt*(2c/3)))
                ps = matvec(None, v2)
                nc.vector.tensor_sub(d12[:], s1[:], s2[:])
                nc.vector.scalar_tensor_tensor(ac[:], C23, dt, a[:], Alu.mult, Alu.add)
                nc.vector.scalar_tensor_tensor(u[:], ps[:], dt, ac[:], Alu.mult, Alu.add)
                nc.scalar.activation(s3[:], u[:], Act.Tanh)
                nc.vector.tensor_add(v3[:], d12[:], s3[:])

                # round 3: p3 = v3 @ W21 ; s4 = tanh(dt*p3 + (a + dt*c))
                ps = matvec(None, v3)
                nc.vector.tensor_add(qq[:], s2[:], s3[:])
                nc.vector.scalar_tensor_tensor(hh[:], qq[:], 3.0, s1[:], Alu.mult, Alu.add)
                nc.vector.scalar_tensor_tensor(ac[:], CC, dt, a[:], Alu.mult, Alu.add)
                nc.vector.scalar_tensor_tensor(u[:], ps[:], dt, ac[:], Alu.mult, Alu.add)
                nc.scalar.activation(s4[:], u[:], Act.Tanh)
                nc.vector.tensor_add(v4[:], hh[:], s4[:])

                # round 4: p4 = v4 @ W21 ; a' = dt8*p4 + (a + dt8*8c); s1 = tanh(a')
                ps = matvec(None, v4)
                nc.vector.scalar_tensor_tensor(G[:], v4[:], dt8, G[:], Alu.mult, Alu.add)
                nc.vector.scalar_tensor_tensor(ac[:], C8, dt8, a[:], Alu.mult, Alu.add)
                nc.vector.scalar_tensor_tensor(a[:], ps[:], dt8, ac[:], Alu.mult, Alu.add)
                nc.scalar.activation(s1[:], a[:], Act.Tanh)

            # --- yT = G @ W2 + (sum dt) b2 -------------------------------------
            ps = matvec(w2_s, G)
            nc.vector.tensor_add(yt_s[:], ps[:], TB2)
            nc.sync.dma_start(yt_d.rearrange("(t p) -> p t", p=128), yt_s[:])

            # --- regressor: 768->128->256->128->6, LeakyReLU(0.1) --------------
            def lrelu(dst, ps_col, bias):
                # exact leaky relu: max(x, 0.1*x) with x = ps + bias
                x = spool.tile([128, 1], f32, tag="regx")
                nc.vector.tensor_add(x[:], ps_col, bias)
                t01 = spool.tile([128, 1], f32, tag="regt")
                nc.vector.tensor_scalar_mul(t01[:], x[:], 0.1)
                nc.vector.tensor_tensor(dst, x[:], t01[:], Alu.max)

            h1 = spool.tile([128, 1], f32)
            ps = ppool.tile([128, 1], f32, tag="ps")
            for k in range(KT):
                nc.tensor.matmul(
                    ps[:],
                    rw1_s[:, k * 128 : (k + 1) * 128],
                    yt_s[:, k : k + 1],
                    start=(k == 0),
                    stop=(k == KT - 1),
                )
            lrelu(h1[:], ps[:], sm_s[:, 36:37])

            h2 = spool.tile([128, 2], f32)
            for j in range(2):
                ps = ppool.tile([128, 1], f32, tag="ps")
                nc.tensor.matmul(
                    ps[:], rw2_s[:, j * 128 : (j + 1) * 128], h1[:], start=True, stop=True
                )
                lrelu(h2[:, j : j + 1], ps[:], sm_s[:, 37 + j : 38 + j])

            h3 = spool.tile([128, 1], f32)
            ps = ppool.tile([128, 1], f32, tag="ps")
            nc.tensor.matmul(ps[:], rw3_s[:, 0:128], h2[:, 0:1], start=True, stop=False)
            nc.tensor.matmul(ps[:], rw3_s[:, 128:256], h2[:, 1:2], start=False, stop=True)
            lrelu(h3[:], ps[:], sm_s[:, 39:40])

            ps = ppool.tile([128, 1], f32, tag="ps")
            nc.tensor.matmul(ps[0:6, :], rw4_s[:, 0:6], h3[:], start=True, stop=True)
            pose_s = spool.tile([128, 1], f32)
            nc.vector.tensor_add(pose_s[0:6, :], ps[0:6, :], sm_s[0:6, 40:41])
            nc.sync.dma_start(pose_d.rearrange("(p o) -> p o", o=1), pose_s[0:6, :])

    nc.compile()
    return nc


def _get_compiled(dts_key):
    if dts_key not in _CACHE:
        _CACHE[dts_key] = _build(dts_key)
    return _CACHE[dts_key]


def _make_inputs_map(ts, ode_w1, ode_b1, ode_w2, ode_b2,
                     reg_w1, reg_b1, reg_w2, reg_b2, reg_w3, reg_b3, reg_w4, reg_b4):
    ts32 = np.asarray(ts, np.float32)
    dts = ts32[1:] - ts32[:-1]

    W1 = np.asarray(ode_w1, np.float64)
    W2 = np.asarray(ode_w2, np.float64)
    b1 = np.asarray(ode_b1, np.float64)
    b2 = np.asarray(ode_b2, np.float64)
    W21 = np.ascontiguousarray((W2 @ W1).astype(np.float32))
    c = b2 @ W1
    T = float(np.sum(np.asarray(dts, np.float64)))

    def fm(vec):  # 768-vector -> feature-major [128, 6]
        return np.asarray(vec, np.float64).reshape(KT, 128).T

    smalls = np.zeros((128, 48), np.float64)
    smalls[:, 0:6] = fm(b1)
    smalls[:, 6:12] = np.tanh(fm(b1))
    smalls[:, 12:18] = fm(c)
    smalls[:, 18:24] = fm(c * (2.0 / 3.0))
    smalls[:, 24:30] = fm(c * 8.0)
    smalls[:, 30:36] = fm(T * b2)
    smalls[:, 36] = np.asarray(reg_b1, np.float64)
    smalls[:, 37:39] = np.asarray(reg_b2, np.float64).reshape(2, 128).T
    smalls[:, 39] = np.asarray(reg_b3, np.float64)
    smalls[0:6, 40] = np.asarray(reg_b4, np.float64)

    in_map = {
        "w21": W21,
        "w2b": np.ascontiguousarray(np.asarray(ode_w2, np.float32)),
        "rw1": np.ascontiguousarray(np.asarray(reg_w1, np.float32)),
        "rw2": np.ascontiguousarray(np.asarray(reg_w2, np.float32)),
        "rw3": np.ascontiguousarray(np.asarray(reg_w3, np.float32)),
        "rw4": np.ascontiguousarray(np.asarray(reg_w4, np.float32)),
        "smalls": smalls.astype(np.float32),
    }
    return in_map, tuple(float(x) for x in dts)


def kernel(fv, fv_alter, fi, dec, ts,
           ode_w1, ode_b1, ode_w2, ode_b2,
           reg_w1, reg_b1, reg_w2, reg_b2, reg_w3, reg_b3, reg_w4, reg_b4):
    from concourse import bass_utils

    B = np.asarray(fv).shape[0]
    in_map, dts_key = _make_inputs_map(
        ts, ode_w1, ode_b1, ode_w2, ode_b2,
        reg_w1, reg_b1, reg_w2, reg_b2, reg_w3, reg_b3, reg_w4, reg_b4,
    )
    nc = _get_compiled(dts_key)
    res = bass_utils.run_bass_kernel_spmd(
        nc, [in_map] * N_CORES, core_ids=list(range(N_CORES))
    )
    r0 = res.results[0]
    yT = np.broadcast_to(r0["yt"].reshape(1, F), (B, F)).copy()
    pose = np.broadcast_to(r0["pose"].reshape(1, 1, 6), (B, 1, 6)).copy()
    return pose, yT
